# revision 20
# baseline (speedup 1.0000x reference)
"""4-bit column-block-quantized linear on 8 TRN2 cores — fp8 DoubleRow version.

Math:  out[b,o] = scales[o] * (sum_i inp[b,i]*wq[o,i] - zeros[o]*rowsum[b])
where wq nibbles come from packed bytes q[o,j] (j = i//2): even i -> low
nibble, odd i -> high nibble.

Device-side scheme (all O(O*I) work on-device):
  * The packed bytes stream through the PE as float8e4 (e4m3, bias 7).
    Nibble bit patterns 0x0..0xF ARE e4m3 values nibble*2^-9 (subnormals are
    linear), so unpacking is just 2 DVE tensor_scalar ops per 256-row tile:
        l = q & 0x0F0F0F0F          (low nibbles,  pairs even-i activations)
        h = (q >> 4) & 0x0F0F0F0F   (high nibbles, pairs odd-i activations)
    done on uint32 views (single-src ops -> DVE 2x_2p mode).  The 2^9 factor
    is folded into the final scales multiply; with USE_OFFSET the nibbles are
    biased +8 into normal range instead and the bias folds into the rank-1
    correction.
  * Matmuls run fp8 with perf_mode=DoubleRow: one matmul contracts 256 rows
    (two 128-row k-tiles), stationary = activations split hi/lo in e4m3
    (psum rows 0:16 hi, 16:32 lo), moving = the nibble streams.
  * -zeros*rowsum lands via a K=4 bf16 rank-1 correction matmul issued first
    (keeps the PE busy during the initial DMA).
  * Tail per psum block: ACT copies lo rows to SBUF, DVE adds hi rows, DVE
    multiplies by 512*scales, DMA out.

Sharding: column-parallel over out_features (1376 rows/core), inputs
replicated; per-core output [16,1376] gathered on host.
"""

import numpy as np
import ml_dtypes

B = 16
I = 4096
O = 11008
NCORES = 8
OS = O // NCORES          # 1376 out-features per core
HALF = I // 2             # 2048 packed columns (j)
NDKT = 8                  # double-k-tiles of 256 j-rows each
BLKS = [(0, 512), (512, 512), (1024, 352)]  # psum-bank o-blocks

USE_OFFSET = False        # True: bias nibbles +8 (normal-range e4m3) instead
                          # of relying on PE subnormal handling

BF16 = ml_dtypes.bfloat16
FP8 = ml_dtypes.float8_e4m3fn

_CACHE = {}


def _split_bf16(x64):
    hi = x64.astype(BF16)
    lo = (x64 - hi.astype(np.float64)).astype(BF16)
    return hi, lo


def _split_fp8(x64):
    hi = x64.astype(FP8)
    lo = (x64 - hi.astype(np.float64)).astype(FP8)
    return hi, lo


def _build_program():
    import concourse.bacc as bacc
    import concourse.mybir as mybir
    import concourse.tile as tile

    dt = mybir.dt
    op = mybir.AluOpType
    pm = mybir.MatmulPerfMode
    nc = bacc.Bacc("TRN2", target_bir_lowering=False)

    q = nc.dram_tensor("q", [NDKT * 128, 688], dt.uint32, kind="ExternalInput")
    stat = nc.dram_tensor("stat", [128, NDKT * 256], dt.float8e4, kind="ExternalInput")
    corr = nc.dram_tensor("corr", [4, 64 + OS], dt.bfloat16, kind="ExternalInput")
    outs_d = [
        nc.dram_tensor(f"out{i}", [B, n], dt.float32, kind="ExternalOutput")
        for i, (s, n) in enumerate(BLKS)
    ]

    with tile.TileContext(nc) as tc:
        with (
            tc.tile_pool(name="consts", bufs=1) as cpool,
            tc.tile_pool(name="qp", bufs=3) as qpool,
            tc.tile_pool(name="wp", bufs=2) as wpool,
            tc.tile_pool(name="op", bufs=2) as opool,
            tc.tile_pool(name="ps", bufs=1, space="PSUM") as pspool,
        ):
            stat_sb = cpool.tile([128, NDKT * 256], dt.float8e4, name="stat_sb")
            corr_sb = cpool.tile([4, 64 + OS], dt.bfloat16, name="corr_sb")
            corrL_sb = corr_sb[:, 0:64]
            corrR_sb = corr_sb[:, 64 : 64 + OS]

            psums = [
                pspool.tile([64, n], dt.float32, name=f"ps{i}")
                for i, (s, n) in enumerate(BLKS)
            ]

            # tiny consts first so the correction matmuls start immediately;
            # then the first q quad-tile (2 double-k-tiles each)
            nc.sync.dma_start(corr_sb, corr[:, :])
            nc.sync.dma_start(stat_sb, stat[:, :])
            NQD = NDKT // 2
            qts = []
            qtiles = []
            for qd in range(NQD):
                qt = qpool.tile([128, 1376], dt.uint32, name="qt", tag="qt")
                qtiles.append(qt)
                src = q[qd * 256 : (qd + 1) * 256, :].rearrange(
                    "(t p) c -> p t c", t=2
                )
                qt3 = qt.rearrange("p (t c) -> p t c", t=2)
                if qd == 0:
                    nc.sync.dma_start(qt3, src)
                qts.append((qt3, src))
            # rank-1 correction first: PE has work while q tiles stream in
            for i, (s, n) in enumerate(BLKS):
                nc.tensor.matmul(
                    psums[i], corrL_sb, corrR_sb[:, s : s + n],
                    start=True, stop=False,
                )

            def tail(i, s, n):
                t0 = opool.tile([B, n], dt.float32, name="t0", tag=f"t0{i}")
                o = opool.tile([B, n], dt.float32, name="o", tag=f"o{i}")
                # lo-group psum -> sbuf on ACT (one psum read per DVE TT max)
                nc.scalar.activation(
                    t0, psums[i][32:48, :], mybir.ActivationFunctionType.Copy
                )
                nc.vector.tensor_tensor(o, psums[i][0:16, :], t0, op.add)
                # disjoint dram tensors on the ACT HWDGE ring: pipelined
                # and decoupled from the q-ingest ring's drain
                nc.scalar.dma_start(outs_d[i][:, :], o)

            for qd in range(NQD):
                qt3, src = qts[qd]
                qt = qtiles[qd]
                if qd > 0:
                    nc.sync.dma_start(qt3, src)
                lb = wpool.tile([128, 1376], dt.uint32, name="lb", tag="lb")
                hb = wpool.tile([128, 1376], dt.uint32, name="hb", tag="hb")
                if USE_OFFSET:
                    # 0x50|d is e4m3 for 8+d: nibbles biased into normal range
                    nc.vector.tensor_scalar(
                        lb, qt, 0x0F0F0F0F, 0x50505050, op.bitwise_and, op.bitwise_or
                    )
                    nc.vector.tensor_scalar(
                        hb, qt, 4, 0x0F0F0F0F, op.logical_shift_right, op.bitwise_and
                    )
                    nc.vector.tensor_scalar(
                        hb, hb, 0x50505050, None, op.bitwise_or
                    )
                else:
                    nc.vector.tensor_scalar(
                        lb, qt, 0x0F0F0F0F, None, op.bitwise_and
                    )
                    nc.vector.tensor_scalar(
                        hb, qt, 4, 0x0F0F0F0F, op.logical_shift_right, op.bitwise_and
                    )
                lb8 = lb.bitcast(dt.float8e4)
                hb8 = hb.bitcast(dt.float8e4)
                for j in range(2):
                    d = 2 * qd + j
                    # byte pairs (groupA[n], groupB[n]) host-interleaved
                    lbd = lb8[:, j * 2752 : (j + 1) * 2752].rearrange(
                        "p (n g) -> p g n", g=2
                    )
                    hbd = hb8[:, j * 2752 : (j + 1) * 2752].rearrange(
                        "p (n g) -> p g n", g=2
                    )
                    sA = stat_sb[:, d * 256 : d * 256 + 128].rearrange(
                        "p (g m) -> p g m", g=2
                    )
                    sB = stat_sb[:, d * 256 + 128 : d * 256 + 256].rearrange(
                        "p (g m) -> p g m", g=2
                    )
                    if d < NDKT - 1:
                        for i, (s, n) in enumerate(BLKS):
                            nc.tensor.matmul(
                                psums[i], sA, lbd[:, :, s : s + n],
                                start=False, stop=False, perf_mode=pm.DoubleRow,
                            )
                        for i, (s, n) in enumerate(BLKS):
                            nc.tensor.matmul(
                                psums[i], sB, hbd[:, :, s : s + n],
                                start=False, stop=False, perf_mode=pm.DoubleRow,
                            )
                    else:
                        # last dkt: finish + drain blocks one at a time so the
                        # tails overlap the remaining matmuls
                        for i, (s, n) in enumerate(BLKS):
                            nc.tensor.matmul(
                                psums[i], sA, lbd[:, :, s : s + n],
                                start=False, stop=False, perf_mode=pm.DoubleRow,
                            )
                            nc.tensor.matmul(
                                psums[i], sB, hbd[:, :, s : s + n],
                                start=False, stop=True, perf_mode=pm.DoubleRow,
                            )
                            tail(i, s, n)

    nc.finalize()
    return nc


def _get_program():
    if "nc" not in _CACHE:
        _CACHE["nc"] = _build_program()
    return _CACHE["nc"]


def _host_prep(inp, quant_weight, scales, zeros):
    """Build per-core input maps (layout/precision prep, no dequant math)."""
    inp64 = np.asarray(inp, dtype=np.float64)
    a = np.ascontiguousarray(inp64[:, 0::2].T)  # [HALF, B] even-i (pairs l)
    b = np.ascontiguousarray(inp64[:, 1::2].T)  # [HALF, B] odd-i  (pairs h)
    a_hi, a_lo = _split_fp8(a)
    b_hi, b_lo = _split_fp8(b)

    def stat(hi, lo):
        # [HALF,B] -> [128, NDKT*2*64]: per dkt d, group g, cols
        # [hi(16) 0(16) lo(16) 0(16)] of j-rows d*256 + g*128 + p
        # (psum partition slices must be 32-aligned -> hi rows 0:16, lo 32:48)
        z = np.zeros((NDKT, 2, 128, B), dtype=FP8)
        m = np.concatenate(
            [hi.reshape(NDKT, 2, 128, B), z, lo.reshape(NDKT, 2, 128, B), z],
            axis=-1,
        )  # [NDKT, 2, 128, 64]
        return np.ascontiguousarray(
            m.transpose(2, 0, 1, 3).reshape(128, NDKT * 128)
        )

    statA = stat(a_hi, a_lo)
    statB = stat(b_hi, b_lo)
    # merged [128, NDKT*256]: per dkt, statA 128 cols then statB 128 cols
    stat_m = np.ascontiguousarray(
        np.concatenate(
            [statA.reshape(128, NDKT, 128), statB.reshape(128, NDKT, 128)],
            axis=-1,
        ).reshape(128, NDKT * 256)
    )

    rowsum = inp64.sum(axis=1)  # [B]
    rs_hi, rs_lo = _split_bf16(rowsum)
    corrL = np.zeros((4, 64), dtype=BF16)
    # stream values are nibble*2^-9 (subnormal path) or nibble+8 (offset
    # path); psum is scaled by P = 2^-9 or 1 accordingly
    s9 = np.float64(1.0 if USE_OFFSET else 2.0**-9)
    corrL[0, :B] = (rs_hi.astype(np.float64) * s9).astype(BF16)
    corrL[1, :B] = corrL[0, :B]
    corrL[2, :B] = (rs_lo.astype(np.float64) * s9).astype(BF16)
    corrL[3, :B] = corrL[2, :B]

    qw = np.asarray(quant_weight)
    scales = np.asarray(scales, dtype=np.float64).reshape(-1)
    zeros = np.asarray(zeros, dtype=np.float64).reshape(-1)

    in_maps = []
    for cidx in range(NCORES):
        rows = slice(cidx * OS, (cidx + 1) * OS)
        qc = np.ascontiguousarray(qw[rows].astype(np.uint8).T)  # [HALF, OS]
        # byte layout per partition: (gA[c0], gB[c0], gA[c1], gB[c1], ...)
        q_arr = np.ascontiguousarray(
            qc.reshape(NDKT, 2, 128, OS).transpose(0, 2, 3, 1).reshape(
                NDKT * 128, 2 * OS
            )
        ).view(np.uint32)
        z8 = zeros[rows] + (8.0 if USE_OFFSET else 0.0)
        z_hi, z_lo = _split_bf16(z8)
        corr_m = np.zeros((4, 64 + OS), dtype=BF16)
        corr_m[:, :64] = corrL
        corr_m[0, 64:] = -z_hi
        corr_m[1, 64:] = -z_lo
        corr_m[2, 64:] = -z_hi
        corr_m[3, 64:] = -z_lo
        in_maps.append({"q": q_arr, "stat": stat_m, "corr": corr_m})
    return in_maps


def kernel(inp, quant_weight, scales, zeros):
    from concourse.bass_utils import run_bass_kernel_spmd

    nc = _get_program()
    in_maps = _host_prep(inp, quant_weight, scales, zeros)
    res = run_bass_kernel_spmd(nc, in_maps, core_ids=list(range(NCORES)))
    sc = np.asarray(scales, dtype=np.float64).reshape(-1)
    sfac = 1.0 if USE_OFFSET else 512.0
    parts = []
    for c in range(NCORES):
        r = res.results[c]
        o = np.concatenate(
            [r[f"out{i}"] for i in range(len(BLKS))], axis=1
        ).astype(np.float64)
        rows = slice(c * OS, (c + 1) * OS)
        parts.append(o * (sc[rows] * sfac)[None, :])
    out = np.concatenate(parts, axis=1)
    return np.ascontiguousarray(out.astype(np.float32))
